# revision 16
# baseline (speedup 1.0000x reference)
"""Trainium2 Bass kernel for nn_Attention_56169582297517.

ref:  q = primary @ W.T + b            [N,L]
      k = secondary @ W.T + b          [M,L]
      s = relu(q @ k.T)                [N,M]
      s = s / max(||s||_row, 1e-12)
      out = s @ secondary              [N,E]

N=M=8192, E=512, L=128.  Sharding: primary rows split across 8 cores
(1024 rows each); secondary/W/b replicated; each core computes its row
slice independently (row-wise L2 norm is local to N).

Per-core plan (normalization deferred to the very end):
  out_row = (relu(q k^T) @ S)_row / max(norm_row, eps)

Layouts: scores are computed TRANSPOSED (m on partitions, n on free) so
the context matmul can contract m on partitions against natural-layout
secondary chunks.  Row norms (a partition-axis reduction in this layout)
are computed on the PE with a ones-vector matmul over bf16 squares.
The e-contraction projections (q/k) need e on partitions, so W, the
primary block, and secondary are transposed on-chip via PE transposes.

PSUM budget (8 banks): tp(1) + proj(1) + scores(1) + norm(1) + ctx(4).
"""

import numpy as np
from contextlib import ExitStack

import concourse.bass as bass
import concourse.bacc as bacc
import concourse.mybir as mybir
import concourse.tile as tile
from concourse.bass_utils import run_bass_kernel_spmd
from concourse.masks import make_identity

N_CORES = 8
N, M, E, L = 8192, 8192, 512, 128
NLOC = N // N_CORES          # 1024 primary rows per core
P = 128
EC = E // P                  # 4 e-chunks of 128
M_CHUNKS = M // P            # 64 m-chunks of 128
SC = 4                       # m-chunks per load superchunk (512 rows)
N_SUPER = M_CHUNKS // SC     # 16
NG = 512                     # n-group width (psum free dim)
N_GROUPS = NLOC // NG        # 2
NB = NG // P                 # 4 n-blocks of 128 per group
EPS = 1e-12

F32 = mybir.dt.float32
F32R = mybir.dt.float32r
BF16 = mybir.dt.bfloat16
AF = mybir.ActivationFunctionType


def _emit(nc: bass.Bass):
    prim = nc.dram_tensor("primary", [NLOC, E], F32, kind="ExternalInput")
    sec = nc.dram_tensor("secondary", [M, E], F32, kind="ExternalInput")
    w_d = nc.dram_tensor("W", [L, E], F32, kind="ExternalInput")
    b_d = nc.dram_tensor("b", [L], F32, kind="ExternalInput")
    out_d = nc.dram_tensor("out", [NLOC, E], F32, kind="ExternalOutput")

    with tile.TileContext(nc) as tc, ExitStack() as ctx:
        consts = ctx.enter_context(tc.tile_pool(name="consts", bufs=1))
        big = ctx.enter_context(tc.tile_pool(name="big", bufs=1))
        stage = ctx.enter_context(tc.tile_pool(name="stage", bufs=2))
        work = ctx.enter_context(tc.tile_pool(name="work", bufs=3))
        psum = ctx.enter_context(tc.tile_pool(name="psum", bufs=1, space="PSUM"))

        # ---------------- constants ----------------
        ident = consts.tile([P, P], F32)
        make_identity(nc, ident)
        b_sb = consts.tile([P, 1], F32)
        with nc.allow_non_contiguous_dma(reason="128x4B bias load, one-off"):
            nc.sync.dma_start(b_sb, b_d[:].rearrange("(p o) -> p o", o=1))
        ones_f = consts.tile([P, 1], F32)
        nc.gpsimd.memset(ones_f, 1.0)
        one_f = consts.tile([1, 1], F32)
        nc.gpsimd.memset(one_f, 1.0)
        w_sb = consts.tile([P, E], F32)
        nc.sync.dma_start(w_sb, w_d[:])

        # W^T staged as wt[e_in, ec, l]  (fp32r: rounded at write, feeds PE)
        wt = consts.tile([P, EC, P], F32R)
        for e in range(EC):
            tp = psum.tile([P, P], F32, tag="tp", name="tp")
            nc.tensor.transpose(tp, w_sb[:, e * P:(e + 1) * P], ident)
            nc.scalar.copy(wt[:, e, :], tp)

        # ---------------- qT = W @ P_loc^T + b  -> [l, n] ----------------
        qt = big.tile([P, NLOC], F32R)
        for h in range(NLOC // NG):
            pq = psum.tile([P, NG], F32, tag="proj", name="pq")
            for nb4 in range(NB):
                pc = stage.tile([P, E], F32, tag="pchunk", name="pc")
                nc.sync.dma_start(pc, prim[(h * NB + nb4) * P:(h * NB + nb4 + 1) * P, :])
                pt_ps = psum.tile([P, EC, P], F32, tag="tp", name="pt_ps")
                for e in range(EC):
                    nc.tensor.transpose(pt_ps[:, e, :], pc[:, e * P:(e + 1) * P], ident)
                pt_sb = stage.tile([P, EC, P], F32R, tag="pt", name="pt_sb")
                nc.vector.tensor_copy(pt_sb, pt_ps)
                for e in range(EC):
                    nc.tensor.matmul(
                        pq[:, nb4 * P:(nb4 + 1) * P],
                        lhsT=wt[:, e, :],
                        rhs=pt_sb[:, e, :],
                        start=(e == 0),
                        stop=(e == EC - 1),
                    )
            nc.scalar.activation(qt[:, h * NG:(h + 1) * NG], pq, AF.Identity, bias=b_sb)

        # ------------- secondary: resident bf16 copy + kT projection -------------
        s_nat = big.tile([P, M_CHUNKS, E], BF16)   # [m_in, mc, e]
        kt = big.tile([P, M], F32R)                 # [l, m]
        for sc in range(N_SUPER):
            s_f32 = stage.tile([P, SC, E], F32, tag="sstage", name="s_f32")
            nc.sync.dma_start(
                s_f32, sec[sc * SC * P:(sc + 1) * SC * P, :].rearrange("(j p) e -> p j e", p=P)
            )
            nc.vector.tensor_copy(s_nat[:, sc * SC:(sc + 1) * SC, :], s_f32)
            st_sb = stage.tile([P, EC, SC * P], F32R, tag="st", name="st_sb")
            for j in range(SC):
                st_ps = psum.tile([P, EC, P], F32, tag="tp", name="st_ps")
                for e in range(EC):
                    nc.tensor.transpose(st_ps[:, e, :], s_f32[:, j, e * P:(e + 1) * P], ident)
                if j % 2 == 0:
                    nc.scalar.copy(st_sb[:, :, j * P:(j + 1) * P], st_ps)
                else:
                    nc.vector.tensor_copy(st_sb[:, :, j * P:(j + 1) * P], st_ps)
            pk = psum.tile([P, SC * P], F32, tag="proj", name="pk")
            for e in range(EC):
                nc.tensor.matmul(
                    pk,
                    lhsT=wt[:, e, :],
                    rhs=st_sb[:, e, :],
                    start=(e == 0),
                    stop=(e == EC - 1),
                )
            nc.scalar.activation(kt[:, sc * SC * P:(sc + 1) * SC * P], pk, AF.Identity, bias=b_sb)

        # ---------------- main loop: scores^T, norms, context ----------------
        for g in range(N_GROUPS):
            ctx_ps = [
                psum.tile([P, E], F32, tag=f"ctx{jb}", name=f"ctx{jb}") for jb in range(NB)
            ]
            acc = work.tile([P, NG], F32, tag="acc", name="acc", bufs=1)
            nc.gpsimd.memset(acc, 0.0)
            for mc in range(M_CHUNKS):
                sc_ps = psum.tile([P, NG], F32, tag="scores", name="sc_ps", bufs=2)
                nc.tensor.matmul(
                    sc_ps,
                    lhsT=kt[:, mc * P:(mc + 1) * P],
                    rhs=qt[:, g * NG:(g + 1) * NG],
                    start=True,
                    stop=True,
                )
                st_t = work.tile([P, NG], BF16, tag="sT", name="st_t")
                nc.scalar.activation(st_t, sc_ps, AF.Relu)
                sq_t = work.tile([P, NG], BF16, tag="sq", name="sq_t", bufs=2)
                nc.vector.tensor_mul(sq_t, st_t, st_t)
                nc.vector.tensor_add(acc, acc, sq_t)
                for jb in range(NB):
                    nc.tensor.matmul(
                        ctx_ps[jb],
                        lhsT=st_t[:, jb * P:(jb + 1) * P],
                        rhs=s_nat[:, mc, :],
                        start=(mc == 0),
                        stop=(mc == M_CHUNKS - 1),
                    )
            # ------- finalize group: out = ctx / max(sqrt(norm2), eps) -------
            n2_ps = psum.tile([1, NG], F32, tag="scores", name="n2_ps", bufs=2)
            nc.tensor.matmul(n2_ps, lhsT=ones_f, rhs=acc, start=True, stop=True)
            norm2_sb = work.tile([1, NG], F32, tag="n2", name="norm2_sb", bufs=1)
            nc.scalar.copy(norm2_sb, n2_ps)
            nt_ps = psum.tile([P, NB], F32, tag="scores", name="nt_ps", bufs=2)
            for jb in range(NB):
                # [1,128] -> [128,1] via matmul with 1x1 ones (tiny transpose)
                nc.tensor.matmul(
                    nt_ps[:, jb:jb + 1],
                    lhsT=norm2_sb[0:1, jb * P:(jb + 1) * P],
                    rhs=one_f,
                    start=True,
                    stop=True,
                )
            nrm = work.tile([P, NB], F32, tag="nrm", name="nrm", bufs=1)
            nc.scalar.activation(nrm, nt_ps, AF.Sqrt)
            nrm_c = work.tile([P, NB], F32, tag="nrmc", name="nrm_c", bufs=1)
            nc.vector.tensor_scalar_max(nrm_c, nrm, EPS)
            recip = work.tile([P, NB], F32, tag="recip", name="recip", bufs=1)
            nc.vector.reciprocal(recip, nrm_c)
            for jb in range(NB):
                o_sb = work.tile([P, E], F32, tag="osb", name="o_sb", bufs=2)
                nc.scalar.activation(o_sb, ctx_ps[jb], AF.Copy, scale=recip[:, jb:jb + 1])
                r0 = g * NG + jb * P
                nc.sync.dma_start(out_d[r0:r0 + P, :], o_sb)

    return nc


_NC_CACHE = None


def _get_nc():
    global _NC_CACHE
    if _NC_CACHE is None:
        nc = bacc.Bacc("TRN2", target_bir_lowering=False, debug=False)
        _emit(nc)
        nc.finalize()
        _NC_CACHE = nc
    return _NC_CACHE


def run_sharded(inputs, **kw):
    nc = _get_nc()
    prim = np.ascontiguousarray(np.asarray(inputs["primary"], dtype=np.float32))
    sec = np.ascontiguousarray(np.asarray(inputs["secondary"], dtype=np.float32))
    w = np.ascontiguousarray(np.asarray(inputs["W"], dtype=np.float32))
    b = np.ascontiguousarray(np.asarray(inputs["b"], dtype=np.float32))
    assert prim.shape == (N, E) and sec.shape == (M, E)
    assert w.shape == (L, E) and b.shape == (L,)
    in_maps = [
        {
            "primary": prim[i * NLOC:(i + 1) * NLOC],
            "secondary": sec,
            "W": w,
            "b": b,
        }
        for i in range(N_CORES)
    ]
    res = run_bass_kernel_spmd(nc, in_maps, list(range(N_CORES)), **kw)
    out = np.concatenate([res.results[i]["out"] for i in range(N_CORES)], axis=0)
    return out, res


def kernel(**inputs) -> np.ndarray:
    out, _ = run_sharded(inputs)
    return out


# revision 20
# speedup vs baseline: 1.0980x; 1.0980x over previous
"""Trainium2 Bass kernel for nn_Attention_56169582297517.

ref:  q = primary @ W.T + b            [N,L]
      k = secondary @ W.T + b          [M,L]
      s = relu(q @ k.T)                [N,M]
      s = s / max(||s||_row, 1e-12)
      out = s @ secondary              [N,E]

N=M=8192, E=512, L=128.  Sharding: primary rows split across 8 cores
(1024 rows each); secondary/W/b replicated; each core computes its row
slice independently (row-wise L2 norm is local to N).

Per-core plan (normalization deferred to the very end):
  out_row = (relu(q k^T) @ S)_row / max(norm_row, eps)

Layouts: scores are computed TRANSPOSED (m on partitions, n on free) so
the context matmul can contract m on partitions against natural-layout
secondary chunks.  Row norms (a partition-axis reduction in this layout)
are computed on the PE with a ones-vector matmul over bf16 squares.
The e-contraction projections (q/k) need e on partitions, so W, the
primary block, and secondary are transposed on-chip via PE transposes.

PSUM budget (8 banks): tp(1) + proj(1) + scores(1) + norm(1) + ctx(4).
"""

import numpy as np
from contextlib import ExitStack

import concourse.bass as bass
import concourse.bacc as bacc
import concourse.mybir as mybir
import concourse.tile as tile
from concourse.bass_utils import run_bass_kernel_spmd
from concourse.masks import make_identity

N_CORES = 8
N, M, E, L = 8192, 8192, 512, 128
NLOC = N // N_CORES          # 1024 primary rows per core
P = 128
EC = E // P                  # 4 e-chunks of 128
M_CHUNKS = M // P            # 64 m-chunks of 128
SC = 4                       # m-chunks per load superchunk (512 rows)
N_SUPER = M_CHUNKS // SC     # 16
NG = 512                     # n-group width (psum free dim)
N_GROUPS = NLOC // NG        # 2
NB = NG // P                 # 4 n-blocks of 128 per group
EPS = 1e-12

F32 = mybir.dt.float32
F32R = mybir.dt.float32r
BF16 = mybir.dt.bfloat16
AF = mybir.ActivationFunctionType


def _emit(nc: bass.Bass):
    prim = nc.dram_tensor("primary", [NLOC, E], F32, kind="ExternalInput")
    sec = nc.dram_tensor("secondary", [M, E], F32, kind="ExternalInput")
    w_d = nc.dram_tensor("W", [L, E], F32, kind="ExternalInput")
    b_d = nc.dram_tensor("b", [L], F32, kind="ExternalInput")
    out_d = nc.dram_tensor("out", [NLOC, E], F32, kind="ExternalOutput")

    with tile.TileContext(nc) as tc, ExitStack() as ctx:
        consts = ctx.enter_context(tc.tile_pool(name="consts", bufs=1))
        big = ctx.enter_context(tc.tile_pool(name="big", bufs=1))
        stage = ctx.enter_context(tc.tile_pool(name="stage", bufs=2))
        work = ctx.enter_context(tc.tile_pool(name="work", bufs=3))
        psum = ctx.enter_context(tc.tile_pool(name="psum", bufs=1, space="PSUM"))

        # ---------------- constants ----------------
        ident = consts.tile([P, P], F32)
        make_identity(nc, ident)
        ident_bf = consts.tile([P, P], BF16)
        make_identity(nc, ident_bf)
        b_sb = consts.tile([P, 1], F32)
        with nc.allow_non_contiguous_dma(reason="128x4B bias load, one-off"):
            nc.sync.dma_start(b_sb, b_d[:].rearrange("(p o) -> p o", o=1))
        ones_f = consts.tile([P, 1], F32)
        nc.gpsimd.memset(ones_f, 1.0)
        one_f = consts.tile([1, 1], F32)
        nc.gpsimd.memset(one_f, 1.0)
        w_sb = consts.tile([P, E], F32)
        nc.sync.dma_start(w_sb, w_d[:])

        # W^T staged as wt[e_in, ec, l]  (bf16: projections run on the bf16 path)
        wt = consts.tile([P, EC, P], BF16)
        for e in range(EC):
            tp = psum.tile([P, P], F32, tag="tp", name="tp")
            nc.tensor.transpose(tp, w_sb[:, e * P:(e + 1) * P], ident)
            nc.scalar.copy(wt[:, e, :], tp)

        # ---------------- qT = W @ P_loc^T + b  -> [l, n] ----------------
        qt = big.tile([P, NLOC], F32R)
        for h in range(NLOC // NG):
            pq = psum.tile([P, NG], F32, tag="proj", name="pq")
            for nb4 in range(NB):
                pc = stage.tile([P, E], F32, tag="pchunk", name="pc")
                nc.sync.dma_start(pc, prim[(h * NB + nb4) * P:(h * NB + nb4 + 1) * P, :])
                pc_bf = stage.tile([P, E], BF16, tag="pchunk_bf", name="pc_bf")
                nc.vector.tensor_copy(pc_bf, pc)
                pt_ps = psum.tile([P, EC, P], BF16, tag="tp", name="pt_ps")
                for e in range(EC):
                    nc.tensor.transpose(pt_ps[:, e, :], pc_bf[:, e * P:(e + 1) * P], ident_bf)
                pt_sb = stage.tile([P, EC, P], BF16, tag="pt", name="pt_sb")
                nc.scalar.copy(pt_sb, pt_ps)
                for e in range(EC):
                    nc.tensor.matmul(
                        pq[:, nb4 * P:(nb4 + 1) * P],
                        lhsT=wt[:, e, :],
                        rhs=pt_sb[:, e, :],
                        start=(e == 0),
                        stop=(e == EC - 1),
                    )
            nc.scalar.activation(qt[:, h * NG:(h + 1) * NG], pq, AF.Identity, bias=b_sb)

        # ------------- secondary: resident bf16 copy + kT projection -------------
        s_nat = big.tile([P, M_CHUNKS, E], BF16)   # [m_in, mc, e]
        kt = big.tile([P, M], F32R)                 # [l, m]
        for sc in range(N_SUPER):
            s_f32 = stage.tile([P, SC, E], F32, tag="sstage", name="s_f32")
            nc.sync.dma_start(
                s_f32, sec[sc * SC * P:(sc + 1) * SC * P, :].rearrange("(j p) e -> p j e", p=P)
            )
            nc.vector.tensor_copy(s_nat[:, sc * SC:(sc + 1) * SC, :], s_f32)
            st_sb = stage.tile([P, EC, SC * P], BF16, tag="st", name="st_sb")
            for j in range(SC):
                st_ps = psum.tile([P, EC, P], BF16, tag="tp", name="st_ps")
                for e in range(EC):
                    nc.tensor.transpose(
                        st_ps[:, e, :], s_nat[:, sc * SC + j, e * P:(e + 1) * P], ident_bf
                    )
                nc.scalar.copy(st_sb[:, :, j * P:(j + 1) * P], st_ps)
            pk = psum.tile([P, SC * P], F32, tag="proj", name="pk")
            for e in range(EC):
                nc.tensor.matmul(
                    pk,
                    lhsT=wt[:, e, :],
                    rhs=st_sb[:, e, :],
                    start=(e == 0),
                    stop=(e == EC - 1),
                )
            nc.scalar.activation(kt[:, sc * SC * P:(sc + 1) * SC * P], pk, AF.Identity, bias=b_sb)

        # ---------------- main loop: scores^T, norms, context ----------------
        for g in range(N_GROUPS):
            ctx_ps = [
                psum.tile([P, E], F32, tag=f"ctx{jb}", name=f"ctx{jb}") for jb in range(NB)
            ]
            acc = work.tile([P, NG], F32, tag="acc", name="acc", bufs=1)
            nc.gpsimd.memset(acc, 0.0)
            for mc in range(M_CHUNKS):
                sc_ps = psum.tile([P, NG], F32, tag="scores", name="sc_ps", bufs=2)
                nc.tensor.matmul(
                    sc_ps,
                    lhsT=kt[:, mc * P:(mc + 1) * P],
                    rhs=qt[:, g * NG:(g + 1) * NG],
                    start=True,
                    stop=True,
                )
                st_t = work.tile([P, NG], BF16, tag="sT", name="st_t")
                nc.scalar.activation(st_t, sc_ps, AF.Relu)
                sq_t = work.tile([P, NG], BF16, tag="sq", name="sq_t", bufs=2)
                nc.vector.tensor_mul(sq_t, st_t, st_t)
                nc.vector.tensor_add(acc, acc, sq_t)
                for jb in range(NB):
                    nc.tensor.matmul(
                        ctx_ps[jb],
                        lhsT=st_t[:, jb * P:(jb + 1) * P],
                        rhs=s_nat[:, mc, :],
                        start=(mc == 0),
                        stop=(mc == M_CHUNKS - 1),
                    )
            # ------- finalize group: out = ctx / max(sqrt(norm2), eps) -------
            n2_ps = psum.tile([1, NG], F32, tag="scores", name="n2_ps", bufs=2)
            nc.tensor.matmul(n2_ps, lhsT=ones_f, rhs=acc, start=True, stop=True)
            norm2_sb = work.tile([1, NG], F32, tag="n2", name="norm2_sb", bufs=1)
            nc.scalar.copy(norm2_sb, n2_ps)
            nt_ps = psum.tile([P, NB], F32, tag="scores", name="nt_ps", bufs=2)
            for jb in range(NB):
                # [1,128] -> [128,1] via matmul with 1x1 ones (tiny transpose)
                nc.tensor.matmul(
                    nt_ps[:, jb:jb + 1],
                    lhsT=norm2_sb[0:1, jb * P:(jb + 1) * P],
                    rhs=one_f,
                    start=True,
                    stop=True,
                )
            nrm = work.tile([P, NB], F32, tag="nrm", name="nrm", bufs=1)
            nc.scalar.activation(nrm, nt_ps, AF.Sqrt)
            nrm_c = work.tile([P, NB], F32, tag="nrmc", name="nrm_c", bufs=1)
            nc.vector.tensor_scalar_max(nrm_c, nrm, EPS)
            recip = work.tile([P, NB], F32, tag="recip", name="recip", bufs=1)
            nc.vector.reciprocal(recip, nrm_c)
            for jb in range(NB):
                o_sb = work.tile([P, E], F32, tag="osb", name="o_sb", bufs=2)
                nc.scalar.activation(o_sb, ctx_ps[jb], AF.Copy, scale=recip[:, jb:jb + 1])
                r0 = g * NG + jb * P
                nc.sync.dma_start(out_d[r0:r0 + P, :], o_sb)

    return nc


_NC_CACHE = None


def _get_nc():
    global _NC_CACHE
    if _NC_CACHE is None:
        nc = bacc.Bacc("TRN2", target_bir_lowering=False, debug=False)
        _emit(nc)
        nc.finalize()
        _NC_CACHE = nc
    return _NC_CACHE


def run_sharded(inputs, **kw):
    nc = _get_nc()
    prim = np.ascontiguousarray(np.asarray(inputs["primary"], dtype=np.float32))
    sec = np.ascontiguousarray(np.asarray(inputs["secondary"], dtype=np.float32))
    w = np.ascontiguousarray(np.asarray(inputs["W"], dtype=np.float32))
    b = np.ascontiguousarray(np.asarray(inputs["b"], dtype=np.float32))
    assert prim.shape == (N, E) and sec.shape == (M, E)
    assert w.shape == (L, E) and b.shape == (L,)
    in_maps = [
        {
            "primary": prim[i * NLOC:(i + 1) * NLOC],
            "secondary": sec,
            "W": w,
            "b": b,
        }
        for i in range(N_CORES)
    ]
    res = run_bass_kernel_spmd(nc, in_maps, list(range(N_CORES)), **kw)
    out = np.concatenate([res.results[i]["out"] for i in range(N_CORES)], axis=0)
    return out, res


def kernel(**inputs) -> np.ndarray:
    out, _ = run_sharded(inputs)
    return out
